# revision 27
# baseline (speedup 1.0000x reference)
"""Trainium2 Bass kernel for sparse (strided) multi-head attention.

Reference computation (B=2, S=2048, H=1024, NH=16, D=64):
    q = (x @ q_w) * sigmoid(phi); k = x @ k_w; v = x @ v_w   (per-head [S, D])
    scores = q k^T / sqrt(D), masked to allowed[i, j] = (j % 4 == 0) | (|i-j| <= 8)
    out = softmax(scores) @ v;  return concat_heads(out) @ o_w + o_b

Sharding: 8 cores = 2 batches x 4 head-groups (4 heads each). Each core gets
x^T for its batch, column-sliced q/k/v weights, row-sliced o_w, and returns a
partial transposed output F^T = (attn_out_heads @ o_w_slice)^T which the host
sums over head-groups, transposes, and biases.

All matmul operands are fp16 (PSUM accumulation stays fp32): scores are in
[-5.1, 5.1] so exp(scores) <= ~165 fits fp16 with no max-subtraction.

Device algorithm per core (scores computed transposed: [key, query]):
  - Q^T/K^T D-major ([2 c-tiles of 128ch] x S); V and V_s = V[::4] S-major
    with a ones column (col 64) so attn@V also emits softmax denominators.
    V_s is computed directly by PE matmuls over stride-4 token slices of x^T.
  - Sparse structure per head: strided part (512 keys x 2048 queries, no
    mask) + 16 diagonal band tiles [128 keys x 144 queries] covering
    |i-j|<=8 including tile-boundary straddle (multiplicative 0/1 mask on
    DVE kills invalid pairs and k%4==0 keys already counted in strided).
  - exp on ScalarE psum->sbuf (no max subtraction needed).
  - attn@V accumulated in PSUM po [65, 2048] per head; row 64 = softmax
    denominators; normalization: ACT copies row 64 out, GpSimd broadcasts it
    across 64 partitions, DVE divides (psum/sbuf -> fp16 outTs).
  - F^T = o_w_slice^T-contraction against normalized head outputs, fp16 out.
"""

import contextlib
import os
import numpy as np

B, S, H = 2, 2048, 1024
NH, D = 16, 64
PHI = 1.6180339887
STRIDE, LOCAL = 4, 8
HPG = 4              # heads per group (= per core)
GC = HPG * D         # channels per core = 256
NKT = S // 128       # 16 key tiles
NSK = S // STRIDE    # 512 strided keys

_CACHE = {}
LAST_RESULTS = None


def host_masks():
    """Band-tile masks (1 = allowed, 0 = disallowed), fp32 (cast later).

    Generic tile t>=1: at_b[:, t, qi] is query q = 128t-8+qi, key k_local on
    partitions (k = 128t + k_local): allowed iff |qi-8-k_local| <= 8 and
    k_local % 4 != 0.
    Tile 0: query q = qi directly: allowed iff |qi-k| <= 8 and k % 4 != 0
    (columns >= 136 are all zero, which also kills overlap with tile 1).
    """
    k = np.arange(128)[:, None]
    qi = np.arange(144)[None, :]
    maskg = np.where((np.abs(qi - 8 - k) <= LOCAL) & (k % STRIDE != 0), 1.0, 0.0)
    mask0 = np.where((np.abs(qi - k) <= LOCAL) & (k % STRIDE != 0), 1.0, 0.0)
    return maskg.astype(np.float32), mask0.astype(np.float32)


def band_q0_n(t):
    """Query-column start and width of band tile t."""
    q0 = 0 if t == 0 else 128 * t - 8
    n = 136 if t == 15 else 144
    return q0, n


def band_segments(t):
    """[(col_lo, col_hi)] of band tile t split at 512-col PSUM bank edges."""
    q0, n = band_q0_n(t)
    segs = []
    s = q0
    while s < q0 + n:
        e = min(q0 + n, (s // 512 + 1) * 512)
        segs.append((s, e))
        s = e
    return segs


def build_nc(loop_n=1, probe=None):
    """Build the per-core Bass program (same NEFF for all 8 cores).

    probe="pe": timing-only variant with exps/masks/normalization removed
    (attention matmuls read stale SBUF) — measures the PE + drain + DMA
    pipeline floor. Output is garbage; never use for correctness.
    """
    import concourse.bass as bass
    import concourse.mybir as mybir
    import concourse.tile as tile
    from concourse import bacc

    f32 = mybir.dt.float32
    DT = mybir.dt.float16
    AF = mybir.ActivationFunctionType

    nc = bacc.Bacc("TRN2", target_bir_lowering=False, debug=False)

    d_xT = nc.dram_tensor("xT", [H, S], DT, kind="ExternalInput")
    d_qw = nc.dram_tensor("qw", [H, GC], DT, kind="ExternalInput")
    d_kw = nc.dram_tensor("kw", [H, GC], DT, kind="ExternalInput")
    d_vw = nc.dram_tensor("vw", [H, GC], DT, kind="ExternalInput")
    d_ow = nc.dram_tensor("ow", [GC, H], DT, kind="ExternalInput")
    d_maskg = nc.dram_tensor("maskg", [128, 144], DT, kind="ExternalInput")
    d_mask0 = nc.dram_tensor("mask0", [128, 144], DT, kind="ExternalInput")
    d_ones = nc.dram_tensor("ones", [128, 80], DT, kind="ExternalInput")
    d_fT = nc.dram_tensor("fT", [H, S], DT, kind="ExternalOutput")

    def mm(out, lhsT, rhs, start, stop, tile_position=None):
        nc.tensor.matmul(out, lhsT, rhs, start=start, stop=stop,
                         skip_group_check=True, tile_position=tile_position)

    with tile.TileContext(nc) as tc:
        with (
            tc.tile_pool(name="consts", bufs=1) as consts,
            tc.tile_pool(name="persist", bufs=1) as persist,
        ):
            sb_maskg = consts.tile([128, 144], DT)
            sb_mask0 = consts.tile([128, 144], DT)
            sb_ow = persist.tile([128, 2, 1024], DT)

            # D-major Q^T / K^T: [128ch (2 heads), c-tile, S]
            sb_QT = persist.tile([128, 2, S], DT)
            sb_KT = persist.tile([128, 2, S], DT)
            sb_KsT = persist.tile([128, 2, NSK], DT)      # strided keys, compacted
            # S-major V with ones col: [128, s-tile, head, 66] (col 64 = 1.0)
            sb_V = persist.tile([128, NKT, HPG, 66], DT)
            # h-major so the V_s gather DMA balances to 3 free dims
            sb_Vs = persist.tile([128, HPG, NSK // 128, 66], DT)
            sb_outTs = persist.tile([128, 2, S], DT)      # c-major head outputs
            sb_qw = persist.tile([128, 8, GC], DT)
            sb_kw = persist.tile([128, 8, GC], DT)
            sb_vw = persist.tile([128, 8, GC], DT)

            # constants: weights / masks / ones columns load once, outside
            # the benchmark loop (they are iteration-invariant)
            nc.sync.dma_start(out=sb_qw[:], in_=d_qw.rearrange("(t p) c -> p t c", p=128))
            nc.sync.dma_start(out=sb_kw[:], in_=d_kw.rearrange("(t p) c -> p t c", p=128))
            nc.sync.dma_start(out=sb_vw[:], in_=d_vw.rearrange("(t p) c -> p t c", p=128))
            nc.sync.dma_start(out=sb_ow[:], in_=d_ow.rearrange("(t p) f -> p t f", p=128))
            nc.sync.dma_start(out=sb_maskg[:], in_=d_maskg[:])
            nc.sync.dma_start(out=sb_mask0[:], in_=d_mask0[:])
            nc.sync.dma_start(
                out=sb_V[:, :, :, 64],
                in_=d_ones[:, 0:64].rearrange("p (t h) -> p t h", h=HPG))
            nc.sync.dma_start(
                out=sb_Vs[:, :, :, 64],
                in_=d_ones[:, 64:80].rearrange("p (h t) -> p h t", h=HPG))

            if probe == "pe":
                nc.gpsimd.memset(sb_outTs[:, :, 0:8], 0)

            loop_cm = tc.For_i(0, loop_n, 1) if loop_n > 1 else contextlib.nullcontext()
            with loop_cm:
             # ---------------- Phase 1: load + QKV projections ----------------
             with (
                tc.tile_pool(name="ph1", bufs=1) as ph1,
                tc.tile_pool(name="psQK", bufs=2, space="PSUM") as psQK,
                tc.tile_pool(name="psV", bufs=4, space="PSUM") as psV,
            ):
                sb_xT = ph1.tile([128, 8, S], DT)
                xt_r = d_xT.rearrange("(t p) s -> t p s", p=128)
                for ht in range(8):
                    nc.sync.dma_start(out=sb_xT[:, ht, :], in_=xt_r[ht])

                def psum_copy(dst, src):
                    # DVE only: in the steady-state loop, phase 1 of iter
                    # i+1 overlaps phase 2 of iter i whose ACT (exp) backlog
                    # is the bottleneck — keep ACT free of drains here
                    nc.vector.tensor_copy(dst, src)

                # Q^T / K^T: per (weight, c-tile, S-half) one [128,1024] psum
                for w_sb, w_out in ((sb_qw, sb_QT), (sb_kw, sb_KT)):
                    for ct in range(2):
                        for q2 in range(2):
                            ps = psQK.tile([128, 1024], f32, tag="qk", name="psqk")
                            for ht in range(8):
                                for u in range(2):
                                    mm(ps[:, 512 * u:512 * (u + 1)],
                                       w_sb[:, ht, 128 * ct:128 * (ct + 1)],
                                       sb_xT[:, ht, 1024 * q2 + 512 * u:1024 * q2 + 512 * (u + 1)],
                                       start=(ht == 0), stop=(ht == 7))
                            psum_copy(w_out[:, ct, 1024 * q2:1024 * (q2 + 1)], ps[:])

                # V (S-major): one [128, GC] psum per s-tile
                for st in range(NKT):
                    ps = psV.tile([128, GC], f32, tag="v", name="psv")
                    for ht in range(8):
                        mm(ps[:], sb_xT[:, ht, 128 * st:128 * (st + 1)],
                           sb_vw[:, ht, :], start=(ht == 0), stop=(ht == 7))
                    psum_copy(sb_V[:, st, :, 0:64],
                              ps.rearrange("p (h d) -> p h d", h=HPG))

                # V_s = V[::4]: partition-strided SBUF->SBUF gather, one DMA
                # per 32-partition destination block (strided key
                # r = 128*sst + 32*m + p' lives at V tile 4*sst + m,
                # partition 4*p')
                for sst in range(4):
                    for m in range(4):
                        nc.gpsimd.dma_start(
                            out=sb_Vs[32 * m:32 * (m + 1), :, sst, 0:64],
                            in_=sb_V[0:128:4, 4 * sst + m, :, 0:64])

                # compact strided K^T
                for ct in range(2):
                    ks = sb_KT[:, ct, :].rearrange("p (r f) -> p r f", f=STRIDE)[:, :, 0]
                    nc.vector.tensor_copy(sb_KsT[:, ct, :], ks)

            # ---------------- Phase 2: attention per head ----------------
            with (
                tc.tile_pool(name="ats", bufs=10) as p_ats,
                tc.tile_pool(name="atb", bufs=3) as p_atb,
                tc.tile_pool(name="srow", bufs=2) as p_srow,
                tc.tile_pool(name="sums", bufs=2) as p_sums,
                tc.tile_pool(name="ost", bufs=2) as p_ost,
                tc.tile_pool(name="psS", bufs=2, space="PSUM") as psS,
                tc.tile_pool(name="psO", bufs=2, space="PSUM") as psO,
            ):
                for cg in range(2):
                    pair = (2 * cg, 2 * cg + 1)
                    # per-pair slices: even head on partitions 0-63, odd on
                    # 64-127 -> adjacent matmuls land in different PE row
                    # groups and execute concurrently (auto tile_position)
                    QTp, KTp, KsTp, ats_p, atb_p = {}, {}, {}, {}, {}
                    for par, h in enumerate(pair):
                        pb = 64 * par
                        QTp[h] = sb_QT[pb:pb + 64, cg, :]
                        KTp[h] = sb_KT[pb:pb + 64, cg, :]
                        KsTp[h] = sb_KsT[pb:pb + 64, cg, :]
                        ats_p[h] = [p_ats.tile([128, S], DT, tag="ats",
                                               name="at_s") for _ in range(4)]
                        atb_p[h] = p_atb.tile([128, NKT, 144], DT, tag="atb",
                                              name="at_b")
                        if probe == "pe":
                            for a in ats_p[h]:
                                nc.gpsimd.memset(a[:, 0:8], 0)
                            nc.gpsimd.memset(atb_p[h][:, 0, 0:8], 0)

                    # strided scores + exp, pair-interleaved
                    for i in range(4):
                        for q2 in range(2):
                            pss = {h: psS.tile([128, 1024], f32, tag="sc",
                                               name="ps_sc") for h in pair}
                            for u in range(2):
                                for h in pair:
                                    mm(pss[h][:, 512 * u:512 * (u + 1)],
                                       KsTp[h][:, 128 * i:128 * (i + 1)],
                                       QTp[h][:, 1024 * q2 + 512 * u:
                                               1024 * q2 + 512 * (u + 1)],
                                       start=True, stop=True)
                            if probe != "pe":
                                for h in pair:
                                    nc.scalar.activation(
                                        ats_p[h][i][:, 1024 * q2:1024 * (q2 + 1)],
                                        pss[h][:], AF.Exp)

                    # band scores, pair-interleaved: 6 tiles per [128,1024]
                    # psum (3 per 512-col bank at offsets 0/144/288)
                    for tl in ([0, 1, 2, 3, 4, 5],
                               [6, 7, 8, 9, 10, 11],
                               [12, 13, 14, 15]):
                        psb = {h: psS.tile([128, 1024], f32, tag="sc",
                                           name="ps_bd") for h in pair}
                        for j, t in enumerate(tl):
                            q0, n = band_q0_n(t)
                            off = 512 * (j // 3) + 144 * (j % 3)
                            for h in pair:
                                mm(psb[h][:, off:off + n],
                                   KTp[h][:, 128 * t:128 * (t + 1)],
                                   QTp[h][:, q0:q0 + n],
                                   start=True, stop=True)
                        t0 = tl[0]
                        if probe != "pe":
                            for h in pair:
                                at_b = atb_p[h]
                                ps = psb[h]
                                g0 = at_b[:, t0:t0 + 3, :].rearrange(
                                    "p a b -> p (a b)")
                                nc.scalar.activation(g0, ps[:, 0:432], AF.Exp)
                                if len(tl) == 6:
                                    g1 = at_b[:, t0 + 3:t0 + 6, :].rearrange(
                                        "p a b -> p (a b)")
                                    nc.scalar.activation(g1, ps[:, 512:944],
                                                         AF.Exp)
                                else:
                                    nc.scalar.activation(at_b[:, 15, 0:136],
                                                         ps[:, 512:648], AF.Exp)

                    # multiplicative band masks on DVE (fp16 2x mode)
                    if probe != "pe":
                        mg = sb_maskg[:]
                        bcast = bass.AP(tensor=mg.tensor, offset=mg.offset,
                                        ap=[list(mg.ap[0]), [0, 14],
                                            list(mg.ap[1])])
                        for h in pair:
                            at_b = atb_p[h]
                            nc.vector.tensor_mul(at_b[:, 0, :], at_b[:, 0, :],
                                                 sb_mask0[:])
                            nc.vector.tensor_mul(at_b[:, 1:15, :],
                                                 at_b[:, 1:15, :], bcast)
                            nc.vector.tensor_mul(at_b[:, 15, 0:136],
                                                 at_b[:, 15, 0:136],
                                                 sb_maskg[:, 0:136])

                    # attn @ [V | 1] + normalization per head
                    for par, h in enumerate(pair):
                      pb = 64 * par
                      at_s, at_b = ats_p[h], atb_p[h]
                      for qh in range(2):
                        qlo = 1024 * qh
                        po = psO.tile([65, 1024], f32, tag="out", name="ps_out")
                        ops = []  # (lo, hi, lhsT, rhs, is_first)
                        for i in range(4):
                            for r in range(2):
                                lo = qlo + 512 * r
                                ops.append((lo, lo + 512,
                                            sb_Vs[:, h, i, 0:65],
                                            at_s[i][:, lo:lo + 512],
                                            i == 0))
                        for t in range(NKT):
                            q0, _ = band_q0_n(t)
                            for (lo, hi) in band_segments(t):
                                if lo >= qlo and hi <= qlo + 1024:
                                    ops.append((lo, hi, sb_V[:, t, h, 0:65],
                                                at_b[:, t, lo - q0:hi - q0],
                                                False))
                        # stop on the last writer of each 512-col region
                        last_for_region = {}
                        for oi, (lo, hi, _, _, _) in enumerate(ops):
                            last_for_region[lo // 512] = oi
                        stops = set(last_for_region.values())
                        for oi, (lo, hi, l, rr, first) in enumerate(ops):
                            mm(po[:, lo - qlo:hi - qlo], l, rr,
                               start=first, stop=(oi in stops))

                        # normalization: row 64 -> p0 -> broadcast -> recip/mul
                        # (HW partition_broadcast only reads partition 0)
                        if probe == "pe":
                            continue
                        srow = p_srow.tile([65, 1024], f32, tag="srow", name="srow")
                        nc.vector.tensor_copy(srow[64:65, :], po[64:65, :])
                        srow0 = p_srow.tile([1, 1024], f32, tag="srow0", name="srow0")
                        nc.gpsimd.dma_start(out=srow0[:], in_=srow[64:65, :])
                        sums = p_sums.tile([64, 1024], f32, tag="sums", name="sums")
                        nc.gpsimd.partition_broadcast(sums[:], srow0[:])
                        nc.vector.reciprocal(sums[:], sums[:])
                        osl = slice(qlo, qlo + 1024)
                        if pb == 0:
                            nc.vector.tensor_mul(
                                sb_outTs[0:64, cg, osl], po[0:64, :], sums[:])
                        else:
                            ost = p_ost.tile([64, 1024], DT, tag="ost", name="ost")
                            nc.vector.tensor_mul(ost[:], po[0:64, :], sums[:])
                            nc.gpsimd.dma_start(out=sb_outTs[64:128, cg, osl],
                                                in_=ost[:])

            # ---------------- Phase 3: F^T = (heads @ o_w)^T ----------------
            with (
                tc.tile_pool(name="stage", bufs=3) as p_stage,
                tc.tile_pool(name="psF", bufs=2, space="PSUM") as psF,
            ):
                cp2 = 0
                for ft in range(8):
                    ps = psF.tile([128, S], f32, tag="ft", name="ps_ft")
                    for ctt in range(2):
                        for qs in range(4):
                            mm(ps[:, 512 * qs:512 * (qs + 1)],
                               sb_ow[:, ctt, 128 * ft:128 * (ft + 1)],
                               sb_outTs[:, ctt, 512 * qs:512 * (qs + 1)],
                               start=(ctt == 0), stop=(ctt == 1))
                    # all-ACT: in the steady-state loop these overlap phase 1
                    # of the next iteration, whose drains keep DVE busy while
                    # ACT is idle (no exps until its phase 2)
                    st = p_stage.tile([128, S], DT, tag="st", name="stg")
                    for half in range(2):
                        sl = slice(1024 * half, 1024 * (half + 1))
                        nc.scalar.copy(st[:, sl], ps[:, sl])
                        cp2 += 1
                    nc.sync.dma_start(
                        out=d_fT.rearrange("(t p) s -> t p s", p=128)[ft],
                        in_=st[:])

    nc.compile()
    return nc


def get_nc():
    if "nc" not in _CACHE:
        _CACHE["nc"] = build_nc()
    return _CACHE["nc"]


def host_inputs(x, q_w, k_w, v_w, o_w, o_b, unity_scale):
    """Per-core input maps (all fp16)."""
    sig = 1.0 / (1.0 + np.exp(-float(np.asarray(unity_scale))))
    qw_eff = (np.asarray(q_w) * (sig / np.sqrt(D))).astype(np.float16)
    xT = np.ascontiguousarray(
        np.asarray(x).transpose(0, 2, 1)).astype(np.float16)
    maskg, mask0 = host_masks()
    k_w = np.asarray(k_w, np.float16)
    v_w = np.asarray(v_w, np.float16)
    o_w = np.asarray(o_w, np.float16)
    in_maps = []
    for c in range(8):
        b, g = c // 4, c % 4
        cs = slice(GC * g, GC * (g + 1))
        in_maps.append({
            "xT": xT[b],
            "qw": np.ascontiguousarray(qw_eff[:, cs]),
            "kw": np.ascontiguousarray(k_w[:, cs]),
            "vw": np.ascontiguousarray(v_w[:, cs]),
            "ow": np.ascontiguousarray(o_w[cs, :]),
            "maskg": maskg.astype(np.float16),
            "mask0": mask0.astype(np.float16),
            "ones": np.ones((128, 80), np.float16),
        })
    return in_maps


def kernel(x, q_w, k_w, v_w, o_w, o_b, unity_scale):
    global LAST_RESULTS
    from concourse.bass_utils import run_bass_kernel_spmd

    nc = get_nc()
    in_maps = host_inputs(x, q_w, k_w, v_w, o_w, o_b, unity_scale)
    res = run_bass_kernel_spmd(nc, in_maps, core_ids=list(range(8)),
                               trace=bool(os.environ.get("KERNEL_TRACE")))
    LAST_RESULTS = res
    out = np.zeros((B, S, H), np.float32)
    for b in range(B):
        acc = np.zeros((H, S), np.float32)
        for g in range(4):
            acc += res.results[4 * b + g]["fT"].astype(np.float32)
        out[b] = acc.T
    out += np.asarray(o_b, np.float32)[None, None, :]
    return out
